# revision 19
# baseline (speedup 1.0000x reference)
"""Trainium2 Bass kernel for the GCN message-passing block (nn_Model_16217796510271).

Contract: kernel(**inputs) takes FULL fp32 inputs (x: [64,243,17,256] + weights)
and returns the FULL fp32 output [64,243,17,256]. Batch axis sharded 8 ways.

v3 design:
- bf16 channels-on-partitions layout, W=486 column windows (NW=4).
- Adjacency fully folded into the PE: y_j accumulates in PSUM as
  U x_j + sum_k V^{A[j,k]} x_k with 2 pre-scaled V copies; no vector mixing.
- Pass 0: ACT drains y+bias to bf16 group tiles, DVE bn_stats per (q,j,w),
  grouped DMA of y to an HBM scratch; x stays resident in SBUF.
- Stats: bn-field algebra on DVE, cross-partition reduce via a ones-vector
  fp32 matmul, one [1,68] AllReduce, broadcast back via a ones matmul.
- Pass 1: reread y; per joint: ACT copy-with-scale (s*y), one wide TT add of
  the residual, one wide in-place TS relu, PE joint-attention with the
  sigmoid gate broadcast via a replicated-w2 stationary, one wide TT gate
  multiply, bf16 output DMA.
"""

import sys

for _p in ("/opt/trn_rl_repo",):
    if _p not in sys.path:
        sys.path.insert(0, _p)

import ml_dtypes
import numpy as np

import concourse.bacc as bacc
import concourse.bass as bass
import concourse.tile as tile
from concourse import mybir
from concourse.bass_utils import run_bass_kernel_spmd

# ---------------------------------------------------------------- constants
CONNECTIONS = {
    10: [9], 9: [8, 10], 8: [7, 9], 14: [15, 8], 15: [16, 14], 11: [12, 8],
    12: [13, 11], 7: [0, 8], 0: [1, 7], 1: [2, 0], 2: [3, 1], 4: [5, 0],
    5: [6, 4], 16: [15], 13: [12], 3: [2], 6: [5],
}
J = 17
C = 256
H = 64
B = 64
T = 243
EPS = 1e-5

NCORES = 8
BPC = B // NCORES
NBT = BPC * T                # 1944
W = 486                      # window width (psum bank = 512 fp32)
NW = NBT // W                # 4
NGLOB = B * T * C            # BN count per joint

F32 = mybir.dt.float32
BF16 = mybir.dt.bfloat16


def _norm_adj() -> np.ndarray:
    adj = np.zeros((J, J), dtype=np.float32)
    for i, ks in CONNECTIONS.items():
        for k in ks:
            adj[i, k] = 1.0
    dinv = adj.sum(-1) ** -0.5
    return (dinv[:, None] * adj * dinv[None, :]).astype(np.float32)


_ADJ = _norm_adj()

# distinct A values -> V variants; EDGES[j] = [(k, variant_idx), ...]
_VARIANTS = sorted({round(float(_ADJ[j, k]), 6)
                    for j, ks in CONNECTIONS.items() for k in ks})
NVAR = len(_VARIANTS)
_VIDX = {v: i for i, v in enumerate(_VARIANTS)}
EDGES = {j: [(k, _VIDX[round(float(_ADJ[j, k]), 6)]) for k in ks]
         for j, ks in CONNECTIONS.items()}

# deg-2 joints whose two edges share one A value take a single pre-mixed
# V matmul: m_j = x_k0 + x_k1 (plain TT add), variant = the common value.
PREMIX = {}     # j -> (k0, k1, variant)
for _j, _ks in CONNECTIONS.items():
    if len(_ks) == 2:
        v0 = _VIDX[round(float(_ADJ[_j, _ks[0]]), 6)]
        v1 = _VIDX[round(float(_ADJ[_j, _ks[1]]), 6)]
        if v0 == v1:
            PREMIX[_j] = (_ks[0], _ks[1], v0)

# joint groups for PSUM pipelining (2 group-q pairs in flight = 8 banks)
_JGROUPS = [[0, 1, 2, 3], [4, 5, 6, 7], [8, 9, 10, 11], [12, 13, 14],
            [15, 16]]


# ---------------------------------------------------------------- device program
def _build_program() -> bass.Bass:
    nc = bacc.Bacc(
        "TRN2",
        target_bir_lowering=False,
        debug=False,
        num_devices=NCORES,
    )
    AF = mybir.ActivationFunctionType
    OP = mybir.AluOpType

    xt = nc.dram_tensor("xt", [NW, 128, 2, J, W], BF16, kind="ExternalInput").ap()
    wu = nc.dram_tensor("wu", [2, 2, 128, 128], BF16, kind="ExternalInput").ap()
    wv = nc.dram_tensor("wv", [NVAR, 2, 2, 128, 128], BF16,
                        kind="ExternalInput").ap()
    wa1 = nc.dram_tensor("wa1", [2, 128, H], BF16, kind="ExternalInput").ap()
    w2r = nc.dram_tensor("w2r", [H, 128], BF16, kind="ExternalInput").ap()
    b2 = nc.dram_tensor("b2", [128, 2, J], F32, kind="ExternalInput").ap()
    bnw = nc.dram_tensor("bnw", [1, J], F32, kind="ExternalInput").ap()
    bnb = nc.dram_tensor("bnb", [1, J], F32, kind="ExternalInput").ap()
    ab1 = nc.dram_tensor("ab1", [H, 1], F32, kind="ExternalInput").ap()
    ab2r = nc.dram_tensor("ab2r", [128, 1], F32, kind="ExternalInput").ap()
    out_t = nc.dram_tensor("out_t", [NW, J, 128, 2, W], BF16,
                           kind="ExternalOutput").ap()

    with tile.TileContext(nc) as tc:
        with (
            tc.tile_pool(name="consts", bufs=1) as consts,
            tc.tile_pool(name="xp", bufs=1) as xp,
            tc.tile_pool(name="mp", bufs=14) as mp,
            tc.tile_pool(name="psp", bufs=8, space="PSUM") as psp,
            tc.tile_pool(name="ygp", bufs=2) as ygp,
            tc.tile_pool(name="ysp", bufs=2) as ysp,
            tc.tile_pool(name="tp", bufs=3) as tp,
            tc.tile_pool(name="obp", bufs=3) as obp,
            tc.tile_pool(name="hp", bufs=2) as hp,
            tc.tile_pool(name="abp", bufs=3) as abp,
            tc.tile_pool(name="stat", bufs=1) as stat,
            tc.tile_pool(name="small", bufs=1) as small,
            tc.tile_pool(name="dram", bufs=1, space="DRAM") as dram,
        ):
            # ---- constants
            wusb = consts.tile([128, 2, 2, 128], BF16)
            nc.sync.dma_start(out=wusb, in_=wu.rearrange("a q k m -> k a q m"))
            wvsb = consts.tile([128, NVAR, 2, 2, 128], BF16)
            nc.sync.dma_start(out=wvsb, in_=wv.rearrange("v a q k m -> k v a q m"))
            wa1sb = consts.tile([128, 2, H], BF16)
            nc.sync.dma_start(out=wa1sb, in_=wa1.rearrange("a k h -> k a h"))
            w2rsb = consts.tile([H, 128], BF16)
            nc.sync.dma_start(out=w2rsb, in_=w2r)
            b2sb = consts.tile([128, 2, J], F32)
            nc.sync.dma_start(out=b2sb, in_=b2)
            bnwsb = consts.tile([1, J], F32)
            nc.sync.dma_start(out=bnwsb, in_=bnw)
            bnbsb = consts.tile([1, J], F32)
            nc.sync.dma_start(out=bnbsb, in_=bnb)
            ab1sb = consts.tile([H, 1], F32)
            nc.sync.dma_start(out=ab1sb, in_=ab1)
            ab2rsb = consts.tile([128, 1], F32)
            nc.sync.dma_start(out=ab2rsb, in_=ab2r)
            ones_col = consts.tile([128, 1], F32)
            nc.vector.memset(ones_col, 1.0)
            ones_row = consts.tile([1, 128], F32)
            nc.vector.memset(ones_row, 1.0)

            # bn_stats out per (q*J+j, w):
            # [cnt_e, mean_e, cnt*var_e, cnt_o, mean_o, cnt*var_o]
            bns = stat.tile([128, 2 * J, NW, 6], F32, name="bns")

            # persistent x tiles, one per window
            x_sb = {}
            for w in range(NW):
                t = xp.tile([128, 2, J, W], BF16, name=f"x_{w}")
                for a in range(2):
                    nc.sync.dma_start(out=t[:, a, 0:9, :],
                                      in_=xt[w][:, a, 0:9, :])
                    nc.sync.dma_start(out=t[:, a, 9:J, :],
                                      in_=xt[w][:, a, 9:J, :])
                x_sb[w] = t

            y_hbm = dram.tile([NW, 2, 128, J, W], BF16)

            # ================= pass 0: y + stats =================
            for w in range(NW):
                # pre-mix tiles (equal-variant deg-2 joints): m = x_k0 + x_k1
                mt = {}
                for j, (k0, k1, v) in PREMIX.items():
                    for a in range(2):
                        m = mp.tile([128, W], BF16, name="m", tag="m")
                        nc.vector.tensor_tensor(
                            out=m,
                            in0=x_sb[w][:, a, k0, :],
                            in1=x_sb[w][:, a, k1, :],
                            op=OP.add,
                        )
                        mt[(a, j)] = m

                for gi, grp in enumerate(_JGROUPS):
                    j0, g = grp[0], len(grp)
                    # moving-operand plan per joint: [(moving_fn, variant)]
                    mvs = {}
                    for j in grp:
                        if j in PREMIX:
                            mvs[j] = [(lambda a, j=j: mt[(a, j)],
                                       PREMIX[j][2])]
                        else:
                            mvs[j] = [
                                (lambda a, k=k: x_sb[w][:, a, k, :], v)
                                for (k, v) in EDGES[j]
                            ]
                    for q in range(2):
                        ps = {}
                        for j in grp:
                            ps[j] = psp.tile([128, W], F32, name="yps",
                                             tag="ps")
                        # U then V^{s}, stationary-grouped
                        for a in range(2):
                            for j in grp:
                                nc.tensor.matmul(
                                    ps[j], wusb[:, a, q, :],
                                    x_sb[w][:, a, j, :],
                                    start=(a == 0), stop=False,
                                )
                        n_ed = sum(len(mvs[j]) for j in grp)
                        done = 0
                        for a in range(2):
                            for v in range(NVAR):
                                for j in grp:
                                    for (mfn, vv) in mvs[j]:
                                        if vv != v:
                                            continue
                                        done += 1
                                        nc.tensor.matmul(
                                            ps[j], wvsb[:, v, a, q, :],
                                            mfn(a),
                                            start=False,
                                            stop=(a == 1 and done == n_ed),
                                        )
                        yg = ygp.tile([128, 4, W], BF16, name="yg", tag="yg")
                        for i, j in enumerate(grp):
                            nc.scalar.activation(
                                out=yg[:, i, :],
                                in_=ps[j],
                                func=AF.Identity,
                                bias=b2sb[:, q, j:j + 1],
                                scale=1.0,
                            )
                            nc.vector.bn_stats(
                                out=bns[:, q * J + j, w],
                                in_=yg[:, i, :],
                            )
                        nc.sync.dma_start(
                            out=y_hbm[w, q][:, j0:j0 + g, :],
                            in_=yg[:, 0:g, :],
                        )

            # prefetch pass-1 y for window 0 (overlaps the stats phase)
            yrd0 = {}
            for q in range(2):
                t = ysp.tile([128, J, W], BF16, name="yrd", tag="ys")
                nc.sync.dma_start(out=t[:, 0:9, :], in_=y_hbm[0, q][:, 0:9, :])
                nc.sync.dma_start(out=t[:, 9:J, :], in_=y_hbm[0, q][:, 9:J, :])
                yrd0[q] = t

            # ---- stats: bn-field algebra, PE partition-reduce, AllReduce
            # sums/243: ms = mean_e + mean_o
            # sumsq: cv_e + cv_o + 243*(mean_e^2 + mean_o^2)
            HW2 = float(W // 2)
            me = bns[:, :, :, 1]
            mo = bns[:, :, :, 4]
            cve = bns[:, :, :, 2]
            cvo = bns[:, :, :, 5]
            ms = stat.tile([128, 2 * J, NW], F32, name="ms")
            nc.vector.tensor_tensor(out=ms, in0=me, in1=mo, op=OP.add)
            cv = stat.tile([128, 2 * J, NW], F32, name="cv")
            nc.vector.tensor_tensor(out=cv, in0=cve, in1=cvo, op=OP.add)
            m2e = stat.tile([128, 2 * J, NW], F32, name="m2e")
            nc.vector.tensor_tensor(out=m2e, in0=me, in1=me, op=OP.mult)
            m2o = stat.tile([128, 2 * J, NW], F32, name="m2o")
            nc.vector.tensor_tensor(out=m2o, in0=mo, in1=mo, op=OP.mult)
            m2s = stat.tile([128, 2 * J, NW], F32, name="m2s")
            nc.vector.tensor_tensor(out=m2s, in0=m2e, in1=m2o, op=OP.add)
            sqc = stat.tile([128, 2 * J, NW], F32, name="sqc")
            nc.vector.scalar_tensor_tensor(
                out=sqc, in0=m2s, scalar=HW2, in1=cv, op0=OP.mult, op1=OP.add)

            comb = stat.tile([128, 4 * J], F32, name="comb")
            for wide, off in ((ms, 0), (sqc, 2 * J)):
                t01 = small.tile([128, 2 * J], F32, name="t01")
                nc.vector.tensor_tensor(out=t01, in0=wide[:, :, 0],
                                        in1=wide[:, :, 1], op=OP.add)
                t23 = small.tile([128, 2 * J], F32, name="t23")
                nc.vector.tensor_tensor(out=t23, in0=wide[:, :, 2],
                                        in1=wide[:, :, 3], op=OP.add)
                nc.vector.tensor_tensor(out=comb[:, off:off + 2 * J],
                                        in0=t01, in1=t23, op=OP.add)

            ps_red = psp.tile([1, 4 * J], F32, name="ps_red", tag="ps")
            nc.tensor.matmul(ps_red, ones_col, comb, start=True, stop=True)
            packed = small.tile([1, 4 * J], F32)
            nc.scalar.copy(out=packed, in_=ps_red)

            cc_in = dram.tile([1, 4 * J], F32)
            cc_out = dram.tile([1, 4 * J], F32)
            nc.sync.dma_start(out=cc_in, in_=packed)
            nc.gpsimd.collective_compute(
                "AllReduce",
                OP.add,
                replica_groups=[list(range(NCORES))],
                ins=[cc_in.opt()],
                outs=[cc_out.opt()],
            )
            stats = small.tile([1, 4 * J], F32)
            nc.sync.dma_start(out=stats, in_=cc_out)

            # mu, var, shat = bnw*rsqrt(var+eps), bhat = bnb - mu*shat
            msum = small.tile([1, J], F32)
            nc.vector.tensor_tensor(out=msum, in0=stats[:, 0:J],
                                    in1=stats[:, J:2 * J], op=OP.add)
            mu = small.tile([1, J], F32)
            nc.vector.tensor_scalar(
                out=mu, in0=msum, scalar1=HW2 / NGLOB, scalar2=None,
                op0=OP.mult)
            qsum = small.tile([1, J], F32)
            nc.vector.tensor_tensor(out=qsum, in0=stats[:, 2 * J:3 * J],
                                    in1=stats[:, 3 * J:4 * J], op=OP.add)
            ey2 = small.tile([1, J], F32)
            nc.vector.tensor_scalar(
                out=ey2, in0=qsum, scalar1=1.0 / NGLOB, scalar2=None,
                op0=OP.mult)
            mu2 = small.tile([1, J], F32)
            nc.vector.tensor_tensor(out=mu2, in0=mu, in1=mu, op=OP.mult)
            var = small.tile([1, J], F32)
            nc.vector.tensor_tensor(out=var, in0=ey2, in1=mu2, op=OP.subtract)
            epssb = small.tile([1, 1], F32)
            nc.vector.memset(epssb, EPS)
            sd = small.tile([1, J], F32)
            nc.scalar.activation(out=sd, in_=var, func=AF.Sqrt, bias=epssb,
                                 scale=1.0)
            rstd = small.tile([1, J], F32)
            nc.vector.reciprocal(out=rstd, in_=sd)
            bc = small.tile([1, 2 * J], F32)
            nc.vector.tensor_tensor(out=bc[:, 0:J], in0=bnwsb, in1=rstd,
                                    op=OP.mult)
            bhat = small.tile([1, J], F32)
            nc.vector.tensor_tensor(out=bhat, in0=mu, in1=bc[:, 0:J],
                                    op=OP.mult)
            nc.vector.tensor_tensor(out=bc[:, J:2 * J], in0=bnbsb, in1=bhat,
                                    op=OP.subtract)

            ps_b = psp.tile([128, 2 * J], F32, name="ps_b", tag="ps")
            nc.tensor.matmul(ps_b, ones_row, bc, start=True, stop=True)
            rep = consts.tile([128, 2 * J], F32)
            nc.scalar.copy(out=rep, in_=ps_b)
            srep = rep[:, 0:J]
            bhrep = rep[:, J:2 * J]

            # ================= pass 1: apply =================
            for w in range(NW):
                if w == 0:
                    yrd = yrd0
                else:
                    yrd = {}
                    for q in range(2):
                        t = ysp.tile([128, J, W], BF16, name="yrd", tag="ys")
                        nc.sync.dma_start(out=t[:, 0:9, :],
                                          in_=y_hbm[w, q][:, 0:9, :])
                        nc.sync.dma_start(out=t[:, 9:J, :],
                                          in_=y_hbm[w, q][:, 9:J, :])
                        yrd[q] = t
                for j in range(J):
                    ob = obp.tile([128, 2, W], BF16, name="ob", tag="ob")
                    for q in range(2):
                        nc.vector.scalar_tensor_tensor(
                            out=ob[:, q, :],
                            in0=yrd[q][:, j, :],
                            scalar=srep[:, j:j + 1],
                            in1=x_sb[w][:, q, j, :],
                            op0=OP.mult,
                            op1=OP.add,
                        )
                    nc.vector.tensor_scalar(
                        out=ob, in0=ob,
                        scalar1=bhrep[:, j:j + 1], scalar2=0.0,
                        op0=OP.add, op1=OP.max)
                    hps = psp.tile([H, W], F32, name="hps", tag="ps")
                    nc.tensor.matmul(hps, wa1sb[:, 0, :], ob[:, 0, :],
                                     start=True, stop=False)
                    nc.tensor.matmul(hps, wa1sb[:, 1, :], ob[:, 1, :],
                                     start=False, stop=True)
                    hbf = hp.tile([H, W], BF16, name="hbf", tag="h")
                    nc.scalar.activation(out=hbf, in_=hps, func=AF.Relu,
                                         bias=ab1sb, scale=1.0)
                    aps = psp.tile([128, W], F32, name="aps", tag="ps")
                    nc.tensor.matmul(aps, w2rsb, hbf, start=True, stop=True)
                    attb = abp.tile([128, W], BF16, name="attb", tag="attb")
                    nc.scalar.activation(out=attb, in_=aps, func=AF.Sigmoid,
                                         bias=ab2rsb, scale=1.0)
                    for q in range(2):
                        nc.vector.tensor_tensor(
                            out=ob[:, q, :], in0=ob[:, q, :], in1=attb,
                            op=OP.mult)
                    nc.sync.dma_start(out=out_t[w, j], in_=ob)

    nc.compile()
    return nc


_CACHE: dict = {}


def _host_inputs(x, U_w, U_b, V_w, V_b, bn_w, bn_b, att_w1, att_b1, att_w2,
                 att_b2):
    f32 = np.float32
    bf16 = ml_dtypes.bfloat16

    def chunks(wT):  # [C(in), M(out)] -> [a, q, 128, 128]
        return np.ascontiguousarray(
            wT.reshape(2, 128, 2, 128).transpose(0, 2, 1, 3))

    uT = np.ascontiguousarray(U_w.T).astype(f32)   # [c_in, c_out]
    vT = np.ascontiguousarray(V_w.T).astype(f32)
    wu = chunks(uT).astype(bf16)
    wv = np.stack([chunks(s * vT) for s in _VARIANTS]).astype(bf16)
    wa1 = np.ascontiguousarray(att_w1.T.reshape(2, 128, H)).astype(bf16)
    w2r = np.ascontiguousarray(
        np.tile(att_w2.T.astype(f32), (1, 128))).astype(bf16)  # [H, 128]
    rowsum = _ADJ.sum(axis=1)
    b2 = (rowsum[None, :] * V_b[:, None] + U_b[:, None]).astype(f32)  # [C, J]
    b2 = np.ascontiguousarray(b2.reshape(2, 128, J).transpose(1, 0, 2))
    bnw = bn_w.reshape(1, J).astype(f32)
    bnb = bn_b.reshape(1, J).astype(f32)
    ab1 = att_b1.reshape(H, 1).astype(f32)
    ab2r = np.ascontiguousarray(
        np.tile(att_b2.reshape(1, 1).astype(f32), (128, 1)))

    shared = dict(wu=wu, wv=wv, wa1=wa1, w2r=w2r, b2=b2, bnw=bnw, bnb=bnb,
                  ab1=ab1, ab2r=ab2r)

    xtf = np.ascontiguousarray(x.transpose(3, 2, 0, 1))  # [C, J, B, T]
    in_maps = []
    for i in range(NCORES):
        xc = xtf[:, :, i * BPC:(i + 1) * BPC, :].reshape(C, J, NBT)
        xc = xc.reshape(2, 128, J, NW, W)
        xc = np.ascontiguousarray(xc.transpose(3, 1, 0, 2, 4)).astype(bf16)
        in_maps.append(dict(xt=xc, **shared))
    return in_maps


def kernel(x, U_w, U_b, V_w, V_b, bn_w, bn_b, att_w1, att_b1, att_w2, att_b2,
           _trace=False):
    x = np.asarray(x, dtype=np.float32)
    args = [np.asarray(a, dtype=np.float32)
            for a in (U_w, U_b, V_w, V_b, bn_w, bn_b, att_w1, att_b1, att_w2,
                      att_b2)]
    in_maps = _host_inputs(x, *args)

    if "nc" not in _CACHE:
        _CACHE["nc"] = _build_program()
    nc = _CACHE["nc"]

    res = run_bass_kernel_spmd(nc, in_maps, list(range(NCORES)), trace=_trace)
    _CACHE["last_results"] = res

    # out_t per core: [NW, J, 128, 2, W] -> [BPC, T, J, C]
    outs = []
    for i in range(NCORES):
        o = res.results[i]["out_t"]                     # bf16
        o = o.transpose(3, 2, 1, 0, 4).reshape(C, J, NBT)
        o = o.transpose(2, 1, 0).reshape(BPC, T, J, C)
        outs.append(o)
    out = np.concatenate(outs, axis=0).astype(np.float32)
    return np.ascontiguousarray(out)


# revision 25
# speedup vs baseline: 1.0731x; 1.0731x over previous
"""Trainium2 Bass kernel for the GCN message-passing block (nn_Model_16217796510271).

Contract: kernel(**inputs) takes FULL fp32 inputs (x: [64,243,17,256] + weights)
and returns the FULL fp32 output [64,243,17,256]. Batch axis sharded 8 ways.

v3 design:
- bf16 channels-on-partitions layout, W=486 column windows (NW=4).
- Adjacency fully folded into the PE: y_j accumulates in PSUM as
  U x_j + sum_k V^{A[j,k]} x_k with 2 pre-scaled V copies; no vector mixing.
- Pass 0: ACT drains y+bias to bf16 group tiles, DVE bn_stats per (q,j,w),
  grouped DMA of y to an HBM scratch; x stays resident in SBUF.
- Stats: bn-field algebra on DVE, cross-partition reduce via a ones-vector
  fp32 matmul, one [1,68] AllReduce, broadcast back via a ones matmul.
- Pass 1: reread y; per joint: ACT copy-with-scale (s*y), one wide TT add of
  the residual, one wide in-place TS relu, PE joint-attention with the
  sigmoid gate broadcast via a replicated-w2 stationary, one wide TT gate
  multiply, bf16 output DMA.
"""

import sys

for _p in ("/opt/trn_rl_repo",):
    if _p not in sys.path:
        sys.path.insert(0, _p)

import ml_dtypes
import numpy as np

import concourse.bacc as bacc
import concourse.bass as bass
import concourse.tile as tile
from concourse import mybir
from concourse.bass_utils import run_bass_kernel_spmd

# ---------------------------------------------------------------- constants
CONNECTIONS = {
    10: [9], 9: [8, 10], 8: [7, 9], 14: [15, 8], 15: [16, 14], 11: [12, 8],
    12: [13, 11], 7: [0, 8], 0: [1, 7], 1: [2, 0], 2: [3, 1], 4: [5, 0],
    5: [6, 4], 16: [15], 13: [12], 3: [2], 6: [5],
}
J = 17
C = 256
H = 64
B = 64
T = 243
EPS = 1e-5

NCORES = 8
BPC = B // NCORES
NBT = BPC * T                # 1944
W = 486                      # window width (psum bank = 512 fp32)
NW = NBT // W                # 4
NGLOB = B * T * C            # BN count per joint

F32 = mybir.dt.float32
BF16 = mybir.dt.bfloat16


def _norm_adj() -> np.ndarray:
    adj = np.zeros((J, J), dtype=np.float32)
    for i, ks in CONNECTIONS.items():
        for k in ks:
            adj[i, k] = 1.0
    dinv = adj.sum(-1) ** -0.5
    return (dinv[:, None] * adj * dinv[None, :]).astype(np.float32)


_ADJ = _norm_adj()

# distinct A values -> V variants; EDGES[j] = [(k, variant_idx), ...]
_VARIANTS = sorted({round(float(_ADJ[j, k]), 6)
                    for j, ks in CONNECTIONS.items() for k in ks})
NVAR = len(_VARIANTS)
_VIDX = {v: i for i, v in enumerate(_VARIANTS)}
EDGES = {j: [(k, _VIDX[round(float(_ADJ[j, k]), 6)]) for k in ks]
         for j, ks in CONNECTIONS.items()}

# deg-2 joints whose two edges share one A value take a single pre-mixed
# V matmul: m_j = x_k0 + x_k1 (plain TT add), variant = the common value.
PREMIX = {}     # j -> (k0, k1, variant)
for _j, _ks in CONNECTIONS.items():
    if len(_ks) == 2:
        v0 = _VIDX[round(float(_ADJ[_j, _ks[0]]), 6)]
        v1 = _VIDX[round(float(_ADJ[_j, _ks[1]]), 6)]
        if v0 == v1:
            PREMIX[_j] = (_ks[0], _ks[1], v0)

# joint groups for PSUM pipelining (2 group-q pairs in flight = 8 banks)
_JGROUPS = [[0, 1, 2, 3], [4, 5, 6, 7], [8, 9, 10, 11], [12, 13, 14],
            [15, 16]]


# ---------------------------------------------------------------- device program
def _build_program() -> bass.Bass:
    nc = bacc.Bacc(
        "TRN2",
        target_bir_lowering=False,
        debug=False,
        num_devices=NCORES,
    )
    AF = mybir.ActivationFunctionType
    OP = mybir.AluOpType

    xt = nc.dram_tensor("xt", [NW, 128, 2, J, W], BF16, kind="ExternalInput").ap()
    wu = nc.dram_tensor("wu", [2, 2, 128, 128], BF16, kind="ExternalInput").ap()
    wv = nc.dram_tensor("wv", [NVAR, 2, 2, 128, 128], BF16,
                        kind="ExternalInput").ap()
    wa1 = nc.dram_tensor("wa1", [2, 128, H], BF16, kind="ExternalInput").ap()
    w2r = nc.dram_tensor("w2r", [H, 128], BF16, kind="ExternalInput").ap()
    b2 = nc.dram_tensor("b2", [128, 2, J], F32, kind="ExternalInput").ap()
    bnw = nc.dram_tensor("bnw", [1, J], F32, kind="ExternalInput").ap()
    bnb = nc.dram_tensor("bnb", [1, J], F32, kind="ExternalInput").ap()
    ab1 = nc.dram_tensor("ab1", [H, 1], F32, kind="ExternalInput").ap()
    ab2r = nc.dram_tensor("ab2r", [128, 1], F32, kind="ExternalInput").ap()
    out_t = nc.dram_tensor("out_t", [NW, J, 128, 2, W], BF16,
                           kind="ExternalOutput").ap()

    with tile.TileContext(nc) as tc:
        with (
            tc.tile_pool(name="consts", bufs=1) as consts,
            tc.tile_pool(name="xp", bufs=1) as xp,
            tc.tile_pool(name="mp", bufs=12) as mp,
            tc.tile_pool(name="psp", bufs=8, space="PSUM") as psp,
            tc.tile_pool(name="ygp", bufs=2) as ygp,
            tc.tile_pool(name="ysp", bufs=2) as ysp,
            tc.tile_pool(name="tp", bufs=3) as tp,
            tc.tile_pool(name="obp", bufs=4) as obp,
            tc.tile_pool(name="hp", bufs=2) as hp,
            tc.tile_pool(name="abp", bufs=4) as abp,
            tc.tile_pool(name="stat", bufs=1) as stat,
            tc.tile_pool(name="small", bufs=1) as small,
            tc.tile_pool(name="dram", bufs=1, space="DRAM") as dram,
        ):
            # ---- constants
            wusb = consts.tile([128, 2, 2, 128], BF16)
            nc.sync.dma_start(out=wusb, in_=wu.rearrange("a q k m -> k a q m"))
            wvsb = consts.tile([128, NVAR, 2, 2, 128], BF16)
            nc.sync.dma_start(out=wvsb, in_=wv.rearrange("v a q k m -> k v a q m"))
            wa1sb = consts.tile([128, 2, H], BF16)
            nc.sync.dma_start(out=wa1sb, in_=wa1.rearrange("a k h -> k a h"))
            w2rsb = consts.tile([H, 128], BF16)
            nc.sync.dma_start(out=w2rsb, in_=w2r)
            b2sb = consts.tile([128, 2, J], F32)
            nc.sync.dma_start(out=b2sb, in_=b2)
            bnwsb = consts.tile([1, J], F32)
            nc.sync.dma_start(out=bnwsb, in_=bnw)
            bnbsb = consts.tile([1, J], F32)
            nc.sync.dma_start(out=bnbsb, in_=bnb)
            ab1sb = consts.tile([H, 1], F32)
            nc.sync.dma_start(out=ab1sb, in_=ab1)
            ab2rsb = consts.tile([128, 1], F32)
            nc.sync.dma_start(out=ab2rsb, in_=ab2r)
            ones_col = consts.tile([128, 1], F32)
            nc.vector.memset(ones_col, 1.0)
            ones_row = consts.tile([1, 128], F32)
            nc.vector.memset(ones_row, 1.0)

            # bn_stats out per (q*J+j, w):
            # [cnt_e, mean_e, cnt*var_e, cnt_o, mean_o, cnt*var_o]
            bns = stat.tile([128, 2 * J, NW, 6], F32, name="bns")

            # persistent x tiles, one per window; 18 split DMAs per window so
            # each window's x lands fast (all 16 queues) in window order
            x_sb = {}
            for w in range(NW):
                t = xp.tile([128, 2, J, W], BF16, name=f"x_{w}")
                for a in range(2):
                    for j0 in range(0, J, 2):
                        j1 = min(j0 + 2, J)
                        nc.sync.dma_start(out=t[:, a, j0:j1, :],
                                          in_=xt[w][:, a, j0:j1, :])
                x_sb[w] = t

            y_hbm = dram.tile([NW, 2, 128, J, W], BF16)

            # ================= pass 0: y + stats =================
            for w in range(NW):
                # pre-mix tiles (equal-variant deg-2 joints): m = x_k0 + x_k1
                mt = {}
                for j, (k0, k1, v) in PREMIX.items():
                    for a in range(2):
                        m = mp.tile([128, W], BF16, name="m", tag="m")
                        nc.vector.tensor_tensor(
                            out=m,
                            in0=x_sb[w][:, a, k0, :],
                            in1=x_sb[w][:, a, k1, :],
                            op=OP.add,
                        )
                        mt[(a, j)] = m

                for gi, grp in enumerate(_JGROUPS):
                    j0, g = grp[0], len(grp)
                    # moving-operand plan per joint: [(moving_fn, variant)]
                    mvs = {}
                    for j in grp:
                        if j in PREMIX:
                            mvs[j] = [(lambda a, j=j: mt[(a, j)],
                                       PREMIX[j][2])]
                        else:
                            mvs[j] = [
                                (lambda a, k=k: x_sb[w][:, a, k, :], v)
                                for (k, v) in EDGES[j]
                            ]
                    for q in range(2):
                        ps = {}
                        for j in grp:
                            ps[j] = psp.tile([128, W], F32, name="yps",
                                             tag="ps")
                        # U then V^{s}, stationary-grouped
                        for a in range(2):
                            for j in grp:
                                nc.tensor.matmul(
                                    ps[j], wusb[:, a, q, :],
                                    x_sb[w][:, a, j, :],
                                    start=(a == 0), stop=False,
                                )
                        n_ed = sum(len(mvs[j]) for j in grp)
                        done = 0
                        for a in range(2):
                            for v in range(NVAR):
                                for j in grp:
                                    for (mfn, vv) in mvs[j]:
                                        if vv != v:
                                            continue
                                        done += 1
                                        nc.tensor.matmul(
                                            ps[j], wvsb[:, v, a, q, :],
                                            mfn(a),
                                            start=False,
                                            stop=(a == 1 and done == n_ed),
                                        )
                        yg = ygp.tile([128, 4, W], BF16, name="yg", tag="yg")
                        for i, j in enumerate(grp):
                            nc.scalar.activation(
                                out=yg[:, i, :],
                                in_=ps[j],
                                func=AF.Identity,
                                bias=b2sb[:, q, j:j + 1],
                                scale=1.0,
                            )
                            nc.vector.bn_stats(
                                out=bns[:, q * J + j, w],
                                in_=yg[:, i, :],
                            )
                        nc.sync.dma_start(
                            out=y_hbm[w, q][:, j0:j0 + g, :],
                            in_=yg[:, 0:g, :],
                        )

            # prefetch pass-1 y for window 0 (overlaps the stats phase)
            yrd0 = {}
            for q in range(2):
                t = ysp.tile([128, J, W], BF16, name="yrd", tag="ys")
                nc.sync.dma_start(out=t[:, 0:9, :], in_=y_hbm[0, q][:, 0:9, :])
                nc.sync.dma_start(out=t[:, 9:J, :], in_=y_hbm[0, q][:, 9:J, :])
                yrd0[q] = t

            # ---- stats: bn-field algebra, PE partition-reduce, AllReduce
            # sums/243: ms = mean_e + mean_o
            # sumsq: cv_e + cv_o + 243*(mean_e^2 + mean_o^2)
            HW2 = float(W // 2)
            me = bns[:, :, :, 1]
            mo = bns[:, :, :, 4]
            cve = bns[:, :, :, 2]
            cvo = bns[:, :, :, 5]
            ms = stat.tile([128, 2 * J, NW], F32, name="ms")
            nc.vector.tensor_tensor(out=ms, in0=me, in1=mo, op=OP.add)
            cv = stat.tile([128, 2 * J, NW], F32, name="cv")
            nc.vector.tensor_tensor(out=cv, in0=cve, in1=cvo, op=OP.add)
            m2e = stat.tile([128, 2 * J, NW], F32, name="m2e")
            nc.vector.tensor_tensor(out=m2e, in0=me, in1=me, op=OP.mult)
            m2o = stat.tile([128, 2 * J, NW], F32, name="m2o")
            nc.vector.tensor_tensor(out=m2o, in0=mo, in1=mo, op=OP.mult)
            m2s = stat.tile([128, 2 * J, NW], F32, name="m2s")
            nc.vector.tensor_tensor(out=m2s, in0=m2e, in1=m2o, op=OP.add)
            sqc = stat.tile([128, 2 * J, NW], F32, name="sqc")
            nc.vector.scalar_tensor_tensor(
                out=sqc, in0=m2s, scalar=HW2, in1=cv, op0=OP.mult, op1=OP.add)

            comb = stat.tile([128, 4 * J], F32, name="comb")
            for wide, off in ((ms, 0), (sqc, 2 * J)):
                t01 = small.tile([128, 2 * J], F32, name="t01")
                nc.vector.tensor_tensor(out=t01, in0=wide[:, :, 0],
                                        in1=wide[:, :, 1], op=OP.add)
                t23 = small.tile([128, 2 * J], F32, name="t23")
                nc.vector.tensor_tensor(out=t23, in0=wide[:, :, 2],
                                        in1=wide[:, :, 3], op=OP.add)
                nc.vector.tensor_tensor(out=comb[:, off:off + 2 * J],
                                        in0=t01, in1=t23, op=OP.add)

            ps_red = psp.tile([1, 4 * J], F32, name="ps_red", tag="ps")
            nc.tensor.matmul(ps_red, ones_col, comb, start=True, stop=True)
            packed = small.tile([1, 4 * J], F32)
            nc.scalar.copy(out=packed, in_=ps_red)

            cc_in = dram.tile([1, 4 * J], F32)
            cc_out = dram.tile([1, 4 * J], F32)
            nc.sync.dma_start(out=cc_in, in_=packed)
            nc.gpsimd.collective_compute(
                "AllReduce",
                OP.add,
                replica_groups=[list(range(NCORES))],
                ins=[cc_in.opt()],
                outs=[cc_out.opt()],
            )
            stats = small.tile([1, 4 * J], F32)
            nc.sync.dma_start(out=stats, in_=cc_out)

            # mu, var, shat = bnw*rsqrt(var+eps), bhat = bnb - mu*shat
            msum = small.tile([1, J], F32)
            nc.vector.tensor_tensor(out=msum, in0=stats[:, 0:J],
                                    in1=stats[:, J:2 * J], op=OP.add)
            mu = small.tile([1, J], F32)
            nc.vector.tensor_scalar(
                out=mu, in0=msum, scalar1=HW2 / NGLOB, scalar2=None,
                op0=OP.mult)
            qsum = small.tile([1, J], F32)
            nc.vector.tensor_tensor(out=qsum, in0=stats[:, 2 * J:3 * J],
                                    in1=stats[:, 3 * J:4 * J], op=OP.add)
            ey2 = small.tile([1, J], F32)
            nc.vector.tensor_scalar(
                out=ey2, in0=qsum, scalar1=1.0 / NGLOB, scalar2=None,
                op0=OP.mult)
            mu2 = small.tile([1, J], F32)
            nc.vector.tensor_tensor(out=mu2, in0=mu, in1=mu, op=OP.mult)
            var = small.tile([1, J], F32)
            nc.vector.tensor_tensor(out=var, in0=ey2, in1=mu2, op=OP.subtract)
            epssb = small.tile([1, 1], F32)
            nc.vector.memset(epssb, EPS)
            sd = small.tile([1, J], F32)
            nc.scalar.activation(out=sd, in_=var, func=AF.Sqrt, bias=epssb,
                                 scale=1.0)
            rstd = small.tile([1, J], F32)
            nc.vector.reciprocal(out=rstd, in_=sd)
            bc = small.tile([1, 2 * J], F32)
            nc.vector.tensor_tensor(out=bc[:, 0:J], in0=bnwsb, in1=rstd,
                                    op=OP.mult)
            bhat = small.tile([1, J], F32)
            nc.vector.tensor_tensor(out=bhat, in0=mu, in1=bc[:, 0:J],
                                    op=OP.mult)
            nc.vector.tensor_tensor(out=bc[:, J:2 * J], in0=bnbsb, in1=bhat,
                                    op=OP.subtract)

            ps_b = psp.tile([128, 2 * J], F32, name="ps_b", tag="ps")
            nc.tensor.matmul(ps_b, ones_row, bc, start=True, stop=True)
            rep = consts.tile([128, 2 * J], F32)
            nc.scalar.copy(out=rep, in_=ps_b)
            srep = rep[:, 0:J]
            bhrep = rep[:, J:2 * J]

            # ================= pass 1: apply =================
            for w in range(NW):
                if w == 0:
                    yrd = yrd0
                else:
                    yrd = {}
                    for q in range(2):
                        t = ysp.tile([128, J, W], BF16, name="yrd", tag="ys")
                        nc.sync.dma_start(out=t[:, 0:9, :],
                                          in_=y_hbm[w, q][:, 0:9, :])
                        nc.sync.dma_start(out=t[:, 9:J, :],
                                          in_=y_hbm[w, q][:, 9:J, :])
                        yrd[q] = t
                # software-pipelined: the gate multiply + store for joint j
                # are emitted 2 joints later so the attention-chain latency
                # overlaps the next joints' DVE work
                pend = {}

                def flush(j):
                    ob, attb = pend.pop(j)
                    for q in range(2):
                        nc.vector.tensor_tensor(
                            out=ob[:, q, :], in0=ob[:, q, :], in1=attb,
                            op=OP.mult)
                    nc.sync.dma_start(out=out_t[w, j], in_=ob)

                for j in range(J):
                    ob = obp.tile([128, 2, W], BF16, name="ob", tag="ob")
                    for q in range(2):
                        nc.vector.scalar_tensor_tensor(
                            out=ob[:, q, :],
                            in0=yrd[q][:, j, :],
                            scalar=srep[:, j:j + 1],
                            in1=x_sb[w][:, q, j, :],
                            op0=OP.mult,
                            op1=OP.add,
                        )
                    nc.vector.tensor_scalar(
                        out=ob, in0=ob,
                        scalar1=bhrep[:, j:j + 1], scalar2=0.0,
                        op0=OP.add, op1=OP.max)
                    hps = psp.tile([H, W], F32, name="hps", tag="ps")
                    nc.tensor.matmul(hps, wa1sb[:, 0, :], ob[:, 0, :],
                                     start=True, stop=False)
                    nc.tensor.matmul(hps, wa1sb[:, 1, :], ob[:, 1, :],
                                     start=False, stop=True)
                    hbf = hp.tile([H, W], BF16, name="hbf", tag="h")
                    nc.scalar.activation(out=hbf, in_=hps, func=AF.Relu,
                                         bias=ab1sb, scale=1.0)
                    aps = psp.tile([128, W], F32, name="aps", tag="ps")
                    nc.tensor.matmul(aps, w2rsb, hbf, start=True, stop=True)
                    attb = abp.tile([128, W], BF16, name="attb", tag="attb")
                    nc.scalar.activation(out=attb, in_=aps, func=AF.Sigmoid,
                                         bias=ab2rsb, scale=1.0)
                    pend[j] = (ob, attb)
                    if j >= 2:
                        flush(j - 2)
                for j in (J - 2, J - 1):
                    flush(j)

    nc.compile()
    return nc


_CACHE: dict = {}


def _host_inputs(x, U_w, U_b, V_w, V_b, bn_w, bn_b, att_w1, att_b1, att_w2,
                 att_b2):
    f32 = np.float32
    bf16 = ml_dtypes.bfloat16

    def chunks(wT):  # [C(in), M(out)] -> [a, q, 128, 128]
        return np.ascontiguousarray(
            wT.reshape(2, 128, 2, 128).transpose(0, 2, 1, 3))

    uT = np.ascontiguousarray(U_w.T).astype(f32)   # [c_in, c_out]
    vT = np.ascontiguousarray(V_w.T).astype(f32)
    wu = chunks(uT).astype(bf16)
    wv = np.stack([chunks(s * vT) for s in _VARIANTS]).astype(bf16)
    wa1 = np.ascontiguousarray(att_w1.T.reshape(2, 128, H)).astype(bf16)
    w2r = np.ascontiguousarray(
        np.tile(att_w2.T.astype(f32), (1, 128))).astype(bf16)  # [H, 128]
    rowsum = _ADJ.sum(axis=1)
    b2 = (rowsum[None, :] * V_b[:, None] + U_b[:, None]).astype(f32)  # [C, J]
    b2 = np.ascontiguousarray(b2.reshape(2, 128, J).transpose(1, 0, 2))
    bnw = bn_w.reshape(1, J).astype(f32)
    bnb = bn_b.reshape(1, J).astype(f32)
    ab1 = att_b1.reshape(H, 1).astype(f32)
    ab2r = np.ascontiguousarray(
        np.tile(att_b2.reshape(1, 1).astype(f32), (128, 1)))

    shared = dict(wu=wu, wv=wv, wa1=wa1, w2r=w2r, b2=b2, bnw=bnw, bnb=bnb,
                  ab1=ab1, ab2r=ab2r)

    xtf = np.ascontiguousarray(x.transpose(3, 2, 0, 1))  # [C, J, B, T]
    in_maps = []
    for i in range(NCORES):
        xc = xtf[:, :, i * BPC:(i + 1) * BPC, :].reshape(C, J, NBT)
        xc = xc.reshape(2, 128, J, NW, W)
        xc = np.ascontiguousarray(xc.transpose(3, 1, 0, 2, 4)).astype(bf16)
        in_maps.append(dict(xt=xc, **shared))
    return in_maps


def kernel(x, U_w, U_b, V_w, V_b, bn_w, bn_b, att_w1, att_b1, att_w2, att_b2,
           _trace=False):
    x = np.asarray(x, dtype=np.float32)
    args = [np.asarray(a, dtype=np.float32)
            for a in (U_w, U_b, V_w, V_b, bn_w, bn_b, att_w1, att_b1, att_w2,
                      att_b2)]
    in_maps = _host_inputs(x, *args)

    if "nc" not in _CACHE:
        _CACHE["nc"] = _build_program()
    nc = _CACHE["nc"]

    res = run_bass_kernel_spmd(nc, in_maps, list(range(NCORES)), trace=_trace)
    _CACHE["last_results"] = res

    # out_t per core: [NW, J, 128, 2, W] -> [BPC, T, J, C]
    outs = []
    for i in range(NCORES):
        o = res.results[i]["out_t"]                     # bf16
        o = o.transpose(3, 2, 1, 0, 4).reshape(C, J, NBT)
        o = o.transpose(2, 1, 0).reshape(BPC, T, J, C)
        outs.append(o)
    out = np.concatenate(outs, axis=0).astype(np.float32)
    return np.ascontiguousarray(out)


# revision 26
# speedup vs baseline: 1.1001x; 1.0252x over previous
"""Trainium2 Bass kernel for the GCN message-passing block (nn_Model_16217796510271).

Contract: kernel(**inputs) takes FULL fp32 inputs (x: [64,243,17,256] + weights)
and returns the FULL fp32 output [64,243,17,256]. Batch axis sharded 8 ways.

v3 design:
- bf16 channels-on-partitions layout, W=486 column windows (NW=4).
- Adjacency fully folded into the PE: y_j accumulates in PSUM as
  U x_j + sum_k V^{A[j,k]} x_k with 2 pre-scaled V copies; no vector mixing.
- Pass 0: ACT drains y+bias to bf16 group tiles, DVE bn_stats per (q,j,w),
  grouped DMA of y to an HBM scratch; x stays resident in SBUF.
- Stats: bn-field algebra on DVE, cross-partition reduce via a ones-vector
  fp32 matmul, one [1,68] AllReduce, broadcast back via a ones matmul.
- Pass 1: reread y; per joint: ACT copy-with-scale (s*y), one wide TT add of
  the residual, one wide in-place TS relu, PE joint-attention with the
  sigmoid gate broadcast via a replicated-w2 stationary, one wide TT gate
  multiply, bf16 output DMA.
"""

import sys

for _p in ("/opt/trn_rl_repo",):
    if _p not in sys.path:
        sys.path.insert(0, _p)

import ml_dtypes
import numpy as np

import concourse.bacc as bacc
import concourse.bass as bass
import concourse.tile as tile
from concourse import mybir
from concourse.bass_utils import run_bass_kernel_spmd

# ---------------------------------------------------------------- constants
CONNECTIONS = {
    10: [9], 9: [8, 10], 8: [7, 9], 14: [15, 8], 15: [16, 14], 11: [12, 8],
    12: [13, 11], 7: [0, 8], 0: [1, 7], 1: [2, 0], 2: [3, 1], 4: [5, 0],
    5: [6, 4], 16: [15], 13: [12], 3: [2], 6: [5],
}
J = 17
C = 256
H = 64
B = 64
T = 243
EPS = 1e-5

NCORES = 8
BPC = B // NCORES
NBT = BPC * T                # 1944
W = 486                      # window width (psum bank = 512 fp32)
NW = NBT // W                # 4
NGLOB = B * T * C            # BN count per joint

F32 = mybir.dt.float32
BF16 = mybir.dt.bfloat16


def _norm_adj() -> np.ndarray:
    adj = np.zeros((J, J), dtype=np.float32)
    for i, ks in CONNECTIONS.items():
        for k in ks:
            adj[i, k] = 1.0
    dinv = adj.sum(-1) ** -0.5
    return (dinv[:, None] * adj * dinv[None, :]).astype(np.float32)


_ADJ = _norm_adj()

# distinct A values -> V variants; EDGES[j] = [(k, variant_idx), ...]
_VARIANTS = sorted({round(float(_ADJ[j, k]), 6)
                    for j, ks in CONNECTIONS.items() for k in ks})
NVAR = len(_VARIANTS)
_VIDX = {v: i for i, v in enumerate(_VARIANTS)}
EDGES = {j: [(k, _VIDX[round(float(_ADJ[j, k]), 6)]) for k in ks]
         for j, ks in CONNECTIONS.items()}

# deg-2 joints whose two edges share one A value take a single pre-mixed
# V matmul: m_j = x_k0 + x_k1 (plain TT add), variant = the common value.
PREMIX = {}     # j -> (k0, k1, variant)
for _j, _ks in CONNECTIONS.items():
    if len(_ks) == 2:
        v0 = _VIDX[round(float(_ADJ[_j, _ks[0]]), 6)]
        v1 = _VIDX[round(float(_ADJ[_j, _ks[1]]), 6)]
        if v0 == v1:
            PREMIX[_j] = (_ks[0], _ks[1], v0)

# joint groups for PSUM pipelining (2 group-q pairs in flight = 8 banks)
_JGROUPS = [[0, 1, 2, 3], [4, 5, 6, 7], [8, 9, 10, 11], [12, 13, 14],
            [15, 16]]


# ---------------------------------------------------------------- device program
def _build_program() -> bass.Bass:
    nc = bacc.Bacc(
        "TRN2",
        target_bir_lowering=False,
        debug=False,
        num_devices=NCORES,
    )
    AF = mybir.ActivationFunctionType
    OP = mybir.AluOpType

    xt = nc.dram_tensor("xt", [NW, 128, 2, J, W], BF16, kind="ExternalInput").ap()
    wu = nc.dram_tensor("wu", [2, 2, 128, 128], BF16, kind="ExternalInput").ap()
    wv = nc.dram_tensor("wv", [NVAR, 2, 2, 128, 128], BF16,
                        kind="ExternalInput").ap()
    wa1 = nc.dram_tensor("wa1", [2, 128, H], BF16, kind="ExternalInput").ap()
    w2r = nc.dram_tensor("w2r", [H, 128], BF16, kind="ExternalInput").ap()
    b2 = nc.dram_tensor("b2", [128, 2, J], F32, kind="ExternalInput").ap()
    bnw = nc.dram_tensor("bnw", [1, J], F32, kind="ExternalInput").ap()
    bnb = nc.dram_tensor("bnb", [1, J], F32, kind="ExternalInput").ap()
    ab1 = nc.dram_tensor("ab1", [H, 1], F32, kind="ExternalInput").ap()
    ab2r = nc.dram_tensor("ab2r", [128, 1], F32, kind="ExternalInput").ap()
    out_t = nc.dram_tensor("out_t", [NW, J, 128, 2, W], BF16,
                           kind="ExternalOutput").ap()

    with tile.TileContext(nc) as tc:
        with (
            tc.tile_pool(name="consts", bufs=1) as consts,
            tc.tile_pool(name="xp", bufs=1) as xp,
            tc.tile_pool(name="mp", bufs=12) as mp,
            tc.tile_pool(name="psp", bufs=8, space="PSUM") as psp,
            tc.tile_pool(name="ygp", bufs=2) as ygp,
            tc.tile_pool(name="ysp", bufs=2) as ysp,
            tc.tile_pool(name="tp", bufs=3) as tp,
            tc.tile_pool(name="obp", bufs=4) as obp,
            tc.tile_pool(name="hp", bufs=2) as hp,
            tc.tile_pool(name="abp", bufs=4) as abp,
            tc.tile_pool(name="stat", bufs=1) as stat,
            tc.tile_pool(name="small", bufs=1) as small,
            tc.tile_pool(name="dram", bufs=1, space="DRAM") as dram,
        ):
            # ---- constants
            wusb = consts.tile([128, 2, 2, 128], BF16)
            nc.sync.dma_start(out=wusb, in_=wu.rearrange("a q k m -> k a q m"))
            wvsb = consts.tile([128, NVAR, 2, 2, 128], BF16)
            nc.sync.dma_start(out=wvsb, in_=wv.rearrange("v a q k m -> k v a q m"))
            wa1sb = consts.tile([128, 2, H], BF16)
            nc.sync.dma_start(out=wa1sb, in_=wa1.rearrange("a k h -> k a h"))
            w2rsb = consts.tile([H, 128], BF16)
            nc.sync.dma_start(out=w2rsb, in_=w2r)
            b2sb = consts.tile([128, 2, J], F32)
            nc.sync.dma_start(out=b2sb, in_=b2)
            bnwsb = consts.tile([1, J], F32)
            nc.sync.dma_start(out=bnwsb, in_=bnw)
            bnbsb = consts.tile([1, J], F32)
            nc.sync.dma_start(out=bnbsb, in_=bnb)
            ab1sb = consts.tile([H, 1], F32)
            nc.sync.dma_start(out=ab1sb, in_=ab1)
            ab2rsb = consts.tile([128, 1], F32)
            nc.sync.dma_start(out=ab2rsb, in_=ab2r)
            ones_col = consts.tile([128, 1], F32)
            nc.vector.memset(ones_col, 1.0)
            ones_row = consts.tile([1, 128], F32)
            nc.vector.memset(ones_row, 1.0)

            # bn_stats out per (q*J+j, w):
            # [cnt_e, mean_e, cnt*var_e, cnt_o, mean_o, cnt*var_o]
            bns = stat.tile([128, 2 * J, NW, 6], F32, name="bns")

            # persistent x tiles, one per window; 18 split DMAs per window so
            # each window's x lands fast (all 16 queues) in window order
            x_sb = {}
            for w in range(NW):
                t = xp.tile([128, 2, J, W], BF16, name=f"x_{w}")
                for a in range(2):
                    nc.sync.dma_start(out=t[:, a, :, :],
                                      in_=xt[w][:, a, :, :])
                x_sb[w] = t

            y_hbm = dram.tile([NW, 2, 128, J, W], BF16)

            # ================= pass 0: y + stats =================
            for w in range(NW):
                # pre-mix tiles (equal-variant deg-2 joints): m = x_k0 + x_k1
                mt = {}
                for j, (k0, k1, v) in PREMIX.items():
                    for a in range(2):
                        m = mp.tile([128, W], BF16, name="m", tag="m")
                        nc.vector.tensor_tensor(
                            out=m,
                            in0=x_sb[w][:, a, k0, :],
                            in1=x_sb[w][:, a, k1, :],
                            op=OP.add,
                        )
                        mt[(a, j)] = m

                for gi, grp in enumerate(_JGROUPS):
                    j0, g = grp[0], len(grp)
                    # moving-operand plan per joint: [(moving_fn, variant)]
                    mvs = {}
                    for j in grp:
                        if j in PREMIX:
                            mvs[j] = [(lambda a, j=j: mt[(a, j)],
                                       PREMIX[j][2])]
                        else:
                            mvs[j] = [
                                (lambda a, k=k: x_sb[w][:, a, k, :], v)
                                for (k, v) in EDGES[j]
                            ]
                    for q in range(2):
                        ps = {}
                        for j in grp:
                            ps[j] = psp.tile([128, W], F32, name="yps",
                                             tag="ps")
                        # U then V^{s}, stationary-grouped
                        for a in range(2):
                            for j in grp:
                                nc.tensor.matmul(
                                    ps[j], wusb[:, a, q, :],
                                    x_sb[w][:, a, j, :],
                                    start=(a == 0), stop=False,
                                )
                        n_ed = sum(len(mvs[j]) for j in grp)
                        done = 0
                        for a in range(2):
                            for v in range(NVAR):
                                for j in grp:
                                    for (mfn, vv) in mvs[j]:
                                        if vv != v:
                                            continue
                                        done += 1
                                        nc.tensor.matmul(
                                            ps[j], wvsb[:, v, a, q, :],
                                            mfn(a),
                                            start=False,
                                            stop=(a == 1 and done == n_ed),
                                        )
                        yg = ygp.tile([128, 4, W], BF16, name="yg", tag="yg")
                        for i, j in enumerate(grp):
                            nc.scalar.activation(
                                out=yg[:, i, :],
                                in_=ps[j],
                                func=AF.Identity,
                                bias=b2sb[:, q, j:j + 1],
                                scale=1.0,
                            )
                            nc.vector.bn_stats(
                                out=bns[:, q * J + j, w],
                                in_=yg[:, i, :],
                            )
                        nc.gpsimd.dma_start(
                            out=y_hbm[w, q][:, j0:j0 + g, :],
                            in_=yg[:, 0:g, :],
                        )

            # prefetch pass-1 y for window 0 (overlaps the stats phase)
            yrd0 = {}
            for q in range(2):
                t = ysp.tile([128, J, W], BF16, name="yrd", tag="ys")
                nc.gpsimd.dma_start(out=t, in_=y_hbm[0, q])
                yrd0[q] = t

            # ---- stats: bn-field algebra, PE partition-reduce, AllReduce
            # sums/243: ms = mean_e + mean_o
            # sumsq: cv_e + cv_o + 243*(mean_e^2 + mean_o^2)
            HW2 = float(W // 2)
            me = bns[:, :, :, 1]
            mo = bns[:, :, :, 4]
            cve = bns[:, :, :, 2]
            cvo = bns[:, :, :, 5]
            ms = stat.tile([128, 2 * J, NW], F32, name="ms")
            nc.vector.tensor_tensor(out=ms, in0=me, in1=mo, op=OP.add)
            cv = stat.tile([128, 2 * J, NW], F32, name="cv")
            nc.vector.tensor_tensor(out=cv, in0=cve, in1=cvo, op=OP.add)
            m2e = stat.tile([128, 2 * J, NW], F32, name="m2e")
            nc.vector.tensor_tensor(out=m2e, in0=me, in1=me, op=OP.mult)
            m2o = stat.tile([128, 2 * J, NW], F32, name="m2o")
            nc.vector.tensor_tensor(out=m2o, in0=mo, in1=mo, op=OP.mult)
            m2s = stat.tile([128, 2 * J, NW], F32, name="m2s")
            nc.vector.tensor_tensor(out=m2s, in0=m2e, in1=m2o, op=OP.add)
            sqc = stat.tile([128, 2 * J, NW], F32, name="sqc")
            nc.vector.scalar_tensor_tensor(
                out=sqc, in0=m2s, scalar=HW2, in1=cv, op0=OP.mult, op1=OP.add)

            comb = stat.tile([128, 4 * J], F32, name="comb")
            for wide, off in ((ms, 0), (sqc, 2 * J)):
                t01 = small.tile([128, 2 * J], F32, name="t01")
                nc.vector.tensor_tensor(out=t01, in0=wide[:, :, 0],
                                        in1=wide[:, :, 1], op=OP.add)
                t23 = small.tile([128, 2 * J], F32, name="t23")
                nc.vector.tensor_tensor(out=t23, in0=wide[:, :, 2],
                                        in1=wide[:, :, 3], op=OP.add)
                nc.vector.tensor_tensor(out=comb[:, off:off + 2 * J],
                                        in0=t01, in1=t23, op=OP.add)

            ps_red = psp.tile([1, 4 * J], F32, name="ps_red", tag="ps")
            nc.tensor.matmul(ps_red, ones_col, comb, start=True, stop=True)
            packed = small.tile([1, 4 * J], F32)
            nc.scalar.copy(out=packed, in_=ps_red)

            cc_in = dram.tile([1, 4 * J], F32)
            cc_out = dram.tile([1, 4 * J], F32)
            nc.sync.dma_start(out=cc_in, in_=packed)
            nc.gpsimd.collective_compute(
                "AllReduce",
                OP.add,
                replica_groups=[list(range(NCORES))],
                ins=[cc_in.opt()],
                outs=[cc_out.opt()],
            )
            stats = small.tile([1, 4 * J], F32)
            nc.sync.dma_start(out=stats, in_=cc_out)

            # mu, var, shat = bnw*rsqrt(var+eps), bhat = bnb - mu*shat
            msum = small.tile([1, J], F32)
            nc.vector.tensor_tensor(out=msum, in0=stats[:, 0:J],
                                    in1=stats[:, J:2 * J], op=OP.add)
            mu = small.tile([1, J], F32)
            nc.vector.tensor_scalar(
                out=mu, in0=msum, scalar1=HW2 / NGLOB, scalar2=None,
                op0=OP.mult)
            qsum = small.tile([1, J], F32)
            nc.vector.tensor_tensor(out=qsum, in0=stats[:, 2 * J:3 * J],
                                    in1=stats[:, 3 * J:4 * J], op=OP.add)
            ey2 = small.tile([1, J], F32)
            nc.vector.tensor_scalar(
                out=ey2, in0=qsum, scalar1=1.0 / NGLOB, scalar2=None,
                op0=OP.mult)
            mu2 = small.tile([1, J], F32)
            nc.vector.tensor_tensor(out=mu2, in0=mu, in1=mu, op=OP.mult)
            var = small.tile([1, J], F32)
            nc.vector.tensor_tensor(out=var, in0=ey2, in1=mu2, op=OP.subtract)
            epssb = small.tile([1, 1], F32)
            nc.vector.memset(epssb, EPS)
            sd = small.tile([1, J], F32)
            nc.scalar.activation(out=sd, in_=var, func=AF.Sqrt, bias=epssb,
                                 scale=1.0)
            rstd = small.tile([1, J], F32)
            nc.vector.reciprocal(out=rstd, in_=sd)
            bc = small.tile([1, 2 * J], F32)
            nc.vector.tensor_tensor(out=bc[:, 0:J], in0=bnwsb, in1=rstd,
                                    op=OP.mult)
            bhat = small.tile([1, J], F32)
            nc.vector.tensor_tensor(out=bhat, in0=mu, in1=bc[:, 0:J],
                                    op=OP.mult)
            nc.vector.tensor_tensor(out=bc[:, J:2 * J], in0=bnbsb, in1=bhat,
                                    op=OP.subtract)

            ps_b = psp.tile([128, 2 * J], F32, name="ps_b", tag="ps")
            nc.tensor.matmul(ps_b, ones_row, bc, start=True, stop=True)
            rep = consts.tile([128, 2 * J], F32)
            nc.scalar.copy(out=rep, in_=ps_b)
            srep = rep[:, 0:J]
            bhrep = rep[:, J:2 * J]

            # ================= pass 1: apply =================
            for w in range(NW):
                if w == 0:
                    yrd = yrd0
                else:
                    yrd = {}
                    for q in range(2):
                        t = ysp.tile([128, J, W], BF16, name="yrd", tag="ys")
                        nc.sync.dma_start(out=t[:, 0:9, :],
                                          in_=y_hbm[w, q][:, 0:9, :])
                        nc.sync.dma_start(out=t[:, 9:J, :],
                                          in_=y_hbm[w, q][:, 9:J, :])
                        yrd[q] = t
                # software-pipelined: the gate multiply + store for joint j
                # are emitted 2 joints later so the attention-chain latency
                # overlaps the next joints' DVE work
                pend = {}

                def flush(j):
                    ob, attb = pend.pop(j)
                    for q in range(2):
                        nc.vector.tensor_tensor(
                            out=ob[:, q, :], in0=ob[:, q, :], in1=attb,
                            op=OP.mult)
                    nc.gpsimd.dma_start(out=out_t[w, j], in_=ob)

                for j in range(J):
                    ob = obp.tile([128, 2, W], BF16, name="ob", tag="ob")
                    for q in range(2):
                        nc.vector.scalar_tensor_tensor(
                            out=ob[:, q, :],
                            in0=yrd[q][:, j, :],
                            scalar=srep[:, j:j + 1],
                            in1=x_sb[w][:, q, j, :],
                            op0=OP.mult,
                            op1=OP.add,
                        )
                    nc.vector.tensor_scalar(
                        out=ob, in0=ob,
                        scalar1=bhrep[:, j:j + 1], scalar2=0.0,
                        op0=OP.add, op1=OP.max)
                    hps = psp.tile([H, W], F32, name="hps", tag="ps")
                    nc.tensor.matmul(hps, wa1sb[:, 0, :], ob[:, 0, :],
                                     start=True, stop=False)
                    nc.tensor.matmul(hps, wa1sb[:, 1, :], ob[:, 1, :],
                                     start=False, stop=True)
                    hbf = hp.tile([H, W], BF16, name="hbf", tag="h")
                    nc.scalar.activation(out=hbf, in_=hps, func=AF.Relu,
                                         bias=ab1sb, scale=1.0)
                    aps = psp.tile([128, W], F32, name="aps", tag="ps")
                    nc.tensor.matmul(aps, w2rsb, hbf, start=True, stop=True)
                    attb = abp.tile([128, W], BF16, name="attb", tag="attb")
                    nc.scalar.activation(out=attb, in_=aps, func=AF.Sigmoid,
                                         bias=ab2rsb, scale=1.0)
                    pend[j] = (ob, attb)
                    if j >= 2:
                        flush(j - 2)
                for j in (J - 2, J - 1):
                    flush(j)

    nc.compile()
    return nc


_CACHE: dict = {}


def _host_inputs(x, U_w, U_b, V_w, V_b, bn_w, bn_b, att_w1, att_b1, att_w2,
                 att_b2):
    f32 = np.float32
    bf16 = ml_dtypes.bfloat16

    def chunks(wT):  # [C(in), M(out)] -> [a, q, 128, 128]
        return np.ascontiguousarray(
            wT.reshape(2, 128, 2, 128).transpose(0, 2, 1, 3))

    uT = np.ascontiguousarray(U_w.T).astype(f32)   # [c_in, c_out]
    vT = np.ascontiguousarray(V_w.T).astype(f32)
    wu = chunks(uT).astype(bf16)
    wv = np.stack([chunks(s * vT) for s in _VARIANTS]).astype(bf16)
    wa1 = np.ascontiguousarray(att_w1.T.reshape(2, 128, H)).astype(bf16)
    w2r = np.ascontiguousarray(
        np.tile(att_w2.T.astype(f32), (1, 128))).astype(bf16)  # [H, 128]
    rowsum = _ADJ.sum(axis=1)
    b2 = (rowsum[None, :] * V_b[:, None] + U_b[:, None]).astype(f32)  # [C, J]
    b2 = np.ascontiguousarray(b2.reshape(2, 128, J).transpose(1, 0, 2))
    bnw = bn_w.reshape(1, J).astype(f32)
    bnb = bn_b.reshape(1, J).astype(f32)
    ab1 = att_b1.reshape(H, 1).astype(f32)
    ab2r = np.ascontiguousarray(
        np.tile(att_b2.reshape(1, 1).astype(f32), (128, 1)))

    shared = dict(wu=wu, wv=wv, wa1=wa1, w2r=w2r, b2=b2, bnw=bnw, bnb=bnb,
                  ab1=ab1, ab2r=ab2r)

    xtf = np.ascontiguousarray(x.transpose(3, 2, 0, 1))  # [C, J, B, T]
    in_maps = []
    for i in range(NCORES):
        xc = xtf[:, :, i * BPC:(i + 1) * BPC, :].reshape(C, J, NBT)
        xc = xc.reshape(2, 128, J, NW, W)
        xc = np.ascontiguousarray(xc.transpose(3, 1, 0, 2, 4)).astype(bf16)
        in_maps.append(dict(xt=xc, **shared))
    return in_maps


def kernel(x, U_w, U_b, V_w, V_b, bn_w, bn_b, att_w1, att_b1, att_w2, att_b2,
           _trace=False):
    x = np.asarray(x, dtype=np.float32)
    args = [np.asarray(a, dtype=np.float32)
            for a in (U_w, U_b, V_w, V_b, bn_w, bn_b, att_w1, att_b1, att_w2,
                      att_b2)]
    in_maps = _host_inputs(x, *args)

    if "nc" not in _CACHE:
        _CACHE["nc"] = _build_program()
    nc = _CACHE["nc"]

    res = run_bass_kernel_spmd(nc, in_maps, list(range(NCORES)), trace=_trace)
    _CACHE["last_results"] = res

    # out_t per core: [NW, J, 128, 2, W] -> [BPC, T, J, C]
    outs = []
    for i in range(NCORES):
        o = res.results[i]["out_t"]                     # bf16
        o = o.transpose(3, 2, 1, 0, 4).reshape(C, J, NBT)
        o = o.transpose(2, 1, 0).reshape(BPC, T, J, C)
        outs.append(o)
    out = np.concatenate(outs, axis=0).astype(np.float32)
    return np.ascontiguousarray(out)


# revision 27
# speedup vs baseline: 1.1406x; 1.0369x over previous
"""Trainium2 Bass kernel for the GCN message-passing block (nn_Model_16217796510271).

Contract: kernel(**inputs) takes FULL fp32 inputs (x: [64,243,17,256] + weights)
and returns the FULL fp32 output [64,243,17,256]. Batch axis sharded 8 ways.

v3 design:
- bf16 channels-on-partitions layout, W=486 column windows (NW=4).
- Adjacency fully folded into the PE: y_j accumulates in PSUM as
  U x_j + sum_k V^{A[j,k]} x_k with 2 pre-scaled V copies; no vector mixing.
- Pass 0: ACT drains y+bias to bf16 group tiles, DVE bn_stats per (q,j,w),
  grouped DMA of y to an HBM scratch; x stays resident in SBUF.
- Stats: bn-field algebra on DVE, cross-partition reduce via a ones-vector
  fp32 matmul, one [1,68] AllReduce, broadcast back via a ones matmul.
- Pass 1: reread y; per joint: ACT copy-with-scale (s*y), one wide TT add of
  the residual, one wide in-place TS relu, PE joint-attention with the
  sigmoid gate broadcast via a replicated-w2 stationary, one wide TT gate
  multiply, bf16 output DMA.
"""

import sys

for _p in ("/opt/trn_rl_repo",):
    if _p not in sys.path:
        sys.path.insert(0, _p)

import ml_dtypes
import numpy as np

import concourse.bacc as bacc
import concourse.bass as bass
import concourse.tile as tile
from concourse import mybir
from concourse.bass_utils import run_bass_kernel_spmd

# ---------------------------------------------------------------- constants
CONNECTIONS = {
    10: [9], 9: [8, 10], 8: [7, 9], 14: [15, 8], 15: [16, 14], 11: [12, 8],
    12: [13, 11], 7: [0, 8], 0: [1, 7], 1: [2, 0], 2: [3, 1], 4: [5, 0],
    5: [6, 4], 16: [15], 13: [12], 3: [2], 6: [5],
}
J = 17
C = 256
H = 64
B = 64
T = 243
EPS = 1e-5

NCORES = 8
BPC = B // NCORES
NBT = BPC * T                # 1944
W = 486                      # window width (psum bank = 512 fp32)
NW = NBT // W                # 4
NGLOB = B * T * C            # BN count per joint

F32 = mybir.dt.float32
BF16 = mybir.dt.bfloat16


def _norm_adj() -> np.ndarray:
    adj = np.zeros((J, J), dtype=np.float32)
    for i, ks in CONNECTIONS.items():
        for k in ks:
            adj[i, k] = 1.0
    dinv = adj.sum(-1) ** -0.5
    return (dinv[:, None] * adj * dinv[None, :]).astype(np.float32)


_ADJ = _norm_adj()

# distinct A values -> V variants; EDGES[j] = [(k, variant_idx), ...]
_VARIANTS = sorted({round(float(_ADJ[j, k]), 6)
                    for j, ks in CONNECTIONS.items() for k in ks})
NVAR = len(_VARIANTS)
_VIDX = {v: i for i, v in enumerate(_VARIANTS)}
EDGES = {j: [(k, _VIDX[round(float(_ADJ[j, k]), 6)]) for k in ks]
         for j, ks in CONNECTIONS.items()}

# deg-2 joints whose two edges share one A value take a single pre-mixed
# V matmul: m_j = x_k0 + x_k1 (plain TT add), variant = the common value.
PREMIX = {}     # j -> (k0, k1, variant)
for _j, _ks in CONNECTIONS.items():
    if len(_ks) == 2:
        v0 = _VIDX[round(float(_ADJ[_j, _ks[0]]), 6)]
        v1 = _VIDX[round(float(_ADJ[_j, _ks[1]]), 6)]
        if v0 == v1:
            PREMIX[_j] = (_ks[0], _ks[1], v0)

# joint groups for PSUM pipelining (2 group-q pairs in flight = 8 banks)
_JGROUPS = [[0, 1, 2, 3], [4, 5, 6, 7], [8, 9, 10, 11], [12, 13, 14],
            [15, 16]]


# ---------------------------------------------------------------- device program
def _build_program() -> bass.Bass:
    nc = bacc.Bacc(
        "TRN2",
        target_bir_lowering=False,
        debug=False,
        num_devices=NCORES,
    )
    AF = mybir.ActivationFunctionType
    OP = mybir.AluOpType

    xt = nc.dram_tensor("xt", [NW, 128, 2, J, W], BF16, kind="ExternalInput").ap()
    wu = nc.dram_tensor("wu", [2, 2, 128, 128], BF16, kind="ExternalInput").ap()
    wv = nc.dram_tensor("wv", [NVAR, 2, 2, 128, 128], BF16,
                        kind="ExternalInput").ap()
    wa1 = nc.dram_tensor("wa1", [2, 128, H], BF16, kind="ExternalInput").ap()
    w2r = nc.dram_tensor("w2r", [H, 128], BF16, kind="ExternalInput").ap()
    b2 = nc.dram_tensor("b2", [128, 2, J], F32, kind="ExternalInput").ap()
    bnw = nc.dram_tensor("bnw", [1, J], F32, kind="ExternalInput").ap()
    bnb = nc.dram_tensor("bnb", [1, J], F32, kind="ExternalInput").ap()
    ab1 = nc.dram_tensor("ab1", [H, 1], F32, kind="ExternalInput").ap()
    ab2r = nc.dram_tensor("ab2r", [128, 1], F32, kind="ExternalInput").ap()
    out_t = nc.dram_tensor("out_t", [NW, J, 128, 2, W], BF16,
                           kind="ExternalOutput").ap()

    with tile.TileContext(nc) as tc:
        with (
            tc.tile_pool(name="consts", bufs=1) as consts,
            tc.tile_pool(name="xp", bufs=1) as xp,
            tc.tile_pool(name="mp", bufs=12) as mp,
            tc.tile_pool(name="psp", bufs=8, space="PSUM") as psp,
            tc.tile_pool(name="ygp", bufs=2) as ygp,
            tc.tile_pool(name="ysp", bufs=2) as ysp,
            tc.tile_pool(name="tp", bufs=3) as tp,
            tc.tile_pool(name="obp", bufs=4) as obp,
            tc.tile_pool(name="hp", bufs=2) as hp,
            tc.tile_pool(name="abp", bufs=4) as abp,
            tc.tile_pool(name="stat", bufs=1) as stat,
            tc.tile_pool(name="small", bufs=1) as small,
            tc.tile_pool(name="dram", bufs=1, space="DRAM") as dram,
        ):
            # ---- constants
            wusb = consts.tile([128, 2, 2, 128], BF16)
            nc.sync.dma_start(out=wusb, in_=wu.rearrange("a q k m -> k a q m"))
            wvsb = consts.tile([128, NVAR, 2, 2, 128], BF16)
            nc.sync.dma_start(out=wvsb, in_=wv.rearrange("v a q k m -> k v a q m"))
            wa1sb = consts.tile([128, 2, H], BF16)
            nc.sync.dma_start(out=wa1sb, in_=wa1.rearrange("a k h -> k a h"))
            w2rsb = consts.tile([H, 128], BF16)
            nc.sync.dma_start(out=w2rsb, in_=w2r)
            b2sb = consts.tile([128, 2, J], F32)
            nc.sync.dma_start(out=b2sb, in_=b2)
            bnwsb = consts.tile([1, J], F32)
            nc.sync.dma_start(out=bnwsb, in_=bnw)
            bnbsb = consts.tile([1, J], F32)
            nc.sync.dma_start(out=bnbsb, in_=bnb)
            ab1sb = consts.tile([H, 1], F32)
            nc.sync.dma_start(out=ab1sb, in_=ab1)
            ab2rsb = consts.tile([128, 1], F32)
            nc.sync.dma_start(out=ab2rsb, in_=ab2r)
            ones_col = consts.tile([128, 1], F32)
            nc.vector.memset(ones_col, 1.0)
            ones_row = consts.tile([1, 128], F32)
            nc.vector.memset(ones_row, 1.0)

            # bn_stats out per (q*J+j, w):
            # [cnt_e, mean_e, cnt*var_e, cnt_o, mean_o, cnt*var_o]
            bns = stat.tile([128, 2 * J, NW, 6], F32, name="bns")

            # persistent x tiles, one per window; 18 split DMAs per window so
            # each window's x lands fast (all 16 queues) in window order
            x_sb = {}
            for w in range(NW):
                t = xp.tile([128, 2, J, W], BF16, name=f"x_{w}")
                for a in range(2):
                    nc.sync.dma_start(out=t[:, a, :, :],
                                      in_=xt[w][:, a, :, :])
                x_sb[w] = t

            y_hbm = dram.tile([NW, 2, 128, J, W], BF16)

            # ================= pass 0: y + stats =================
            for w in range(NW):
                # pre-mix tiles (equal-variant deg-2 joints): m = x_k0 + x_k1
                mt = {}
                for j, (k0, k1, v) in PREMIX.items():
                    for a in range(2):
                        m = mp.tile([128, W], BF16, name="m", tag="m")
                        nc.vector.tensor_tensor(
                            out=m,
                            in0=x_sb[w][:, a, k0, :],
                            in1=x_sb[w][:, a, k1, :],
                            op=OP.add,
                        )
                        mt[(a, j)] = m

                for gi, grp in enumerate(_JGROUPS):
                    j0, g = grp[0], len(grp)
                    # moving-operand plan per joint: [(moving_fn, variant)]
                    mvs = {}
                    for j in grp:
                        if j in PREMIX:
                            mvs[j] = [(lambda a, j=j: mt[(a, j)],
                                       PREMIX[j][2])]
                        else:
                            mvs[j] = [
                                (lambda a, k=k: x_sb[w][:, a, k, :], v)
                                for (k, v) in EDGES[j]
                            ]
                    for q in range(2):
                        ps = {}
                        for j in grp:
                            ps[j] = psp.tile([128, W], F32, name="yps",
                                             tag="ps")
                        # U then V^{s}, stationary-grouped
                        for a in range(2):
                            for j in grp:
                                nc.tensor.matmul(
                                    ps[j], wusb[:, a, q, :],
                                    x_sb[w][:, a, j, :],
                                    start=(a == 0), stop=False,
                                )
                        n_ed = sum(len(mvs[j]) for j in grp)
                        done = 0
                        for a in range(2):
                            for v in range(NVAR):
                                for j in grp:
                                    for (mfn, vv) in mvs[j]:
                                        if vv != v:
                                            continue
                                        done += 1
                                        nc.tensor.matmul(
                                            ps[j], wvsb[:, v, a, q, :],
                                            mfn(a),
                                            start=False,
                                            stop=(a == 1 and done == n_ed),
                                        )
                        yg = ygp.tile([128, 4, W], BF16, name="yg", tag="yg")
                        for i, j in enumerate(grp):
                            nc.scalar.activation(
                                out=yg[:, i, :],
                                in_=ps[j],
                                func=AF.Identity,
                                bias=b2sb[:, q, j:j + 1],
                                scale=1.0,
                            )
                            nc.vector.bn_stats(
                                out=bns[:, q * J + j, w],
                                in_=yg[:, i, :],
                            )
                        nc.gpsimd.dma_start(
                            out=y_hbm[w, q][:, j0:j0 + g, :],
                            in_=yg[:, 0:g, :],
                        )

            # prefetch pass-1 y for window 0 (overlaps the stats phase)
            yrd0 = {}
            for q in range(2):
                t = ysp.tile([128, J, W], BF16, name="yrd", tag="ys")
                nc.gpsimd.dma_start(out=t, in_=y_hbm[0, q])
                yrd0[q] = t

            # ---- stats: bn-field algebra, PE partition-reduce, AllReduce
            # sums/243: ms = mean_e + mean_o
            # sumsq: cv_e + cv_o + 243*(mean_e^2 + mean_o^2)
            HW2 = float(W // 2)
            me = bns[:, :, :, 1]
            mo = bns[:, :, :, 4]
            cve = bns[:, :, :, 2]
            cvo = bns[:, :, :, 5]
            ms = stat.tile([128, 2 * J, NW], F32, name="ms")
            nc.vector.tensor_tensor(out=ms, in0=me, in1=mo, op=OP.add)
            cv = stat.tile([128, 2 * J, NW], F32, name="cv")
            nc.vector.tensor_tensor(out=cv, in0=cve, in1=cvo, op=OP.add)
            m2e = stat.tile([128, 2 * J, NW], F32, name="m2e")
            nc.vector.tensor_tensor(out=m2e, in0=me, in1=me, op=OP.mult)
            m2o = stat.tile([128, 2 * J, NW], F32, name="m2o")
            nc.vector.tensor_tensor(out=m2o, in0=mo, in1=mo, op=OP.mult)
            m2s = stat.tile([128, 2 * J, NW], F32, name="m2s")
            nc.vector.tensor_tensor(out=m2s, in0=m2e, in1=m2o, op=OP.add)
            sqc = stat.tile([128, 2 * J, NW], F32, name="sqc")
            nc.vector.scalar_tensor_tensor(
                out=sqc, in0=m2s, scalar=HW2, in1=cv, op0=OP.mult, op1=OP.add)

            comb = stat.tile([128, 4 * J], F32, name="comb")
            for wide, off in ((ms, 0), (sqc, 2 * J)):
                t01 = small.tile([128, 2 * J], F32, name="t01")
                nc.vector.tensor_tensor(out=t01, in0=wide[:, :, 0],
                                        in1=wide[:, :, 1], op=OP.add)
                t23 = small.tile([128, 2 * J], F32, name="t23")
                nc.vector.tensor_tensor(out=t23, in0=wide[:, :, 2],
                                        in1=wide[:, :, 3], op=OP.add)
                nc.vector.tensor_tensor(out=comb[:, off:off + 2 * J],
                                        in0=t01, in1=t23, op=OP.add)

            ps_red = psp.tile([1, 4 * J], F32, name="ps_red", tag="ps")
            nc.tensor.matmul(ps_red, ones_col, comb, start=True, stop=True)
            packed = small.tile([1, 4 * J], F32)
            nc.scalar.copy(out=packed, in_=ps_red)

            cc_in = dram.tile([1, 4 * J], F32)
            cc_out = dram.tile([1, 4 * J], F32)
            nc.sync.dma_start(out=cc_in, in_=packed)
            nc.gpsimd.collective_compute(
                "AllReduce",
                OP.add,
                replica_groups=[list(range(NCORES))],
                ins=[cc_in.opt()],
                outs=[cc_out.opt()],
            )
            stats = small.tile([1, 4 * J], F32)
            nc.sync.dma_start(out=stats, in_=cc_out)

            # mu, var, shat = bnw*rsqrt(var+eps), bhat = bnb - mu*shat
            msum = small.tile([1, J], F32)
            nc.vector.tensor_tensor(out=msum, in0=stats[:, 0:J],
                                    in1=stats[:, J:2 * J], op=OP.add)
            mu = small.tile([1, J], F32)
            nc.vector.tensor_scalar(
                out=mu, in0=msum, scalar1=HW2 / NGLOB, scalar2=None,
                op0=OP.mult)
            qsum = small.tile([1, J], F32)
            nc.vector.tensor_tensor(out=qsum, in0=stats[:, 2 * J:3 * J],
                                    in1=stats[:, 3 * J:4 * J], op=OP.add)
            ey2 = small.tile([1, J], F32)
            nc.vector.tensor_scalar(
                out=ey2, in0=qsum, scalar1=1.0 / NGLOB, scalar2=None,
                op0=OP.mult)
            mu2 = small.tile([1, J], F32)
            nc.vector.tensor_tensor(out=mu2, in0=mu, in1=mu, op=OP.mult)
            var = small.tile([1, J], F32)
            nc.vector.tensor_tensor(out=var, in0=ey2, in1=mu2, op=OP.subtract)
            epssb = small.tile([1, 1], F32)
            nc.vector.memset(epssb, EPS)
            sd = small.tile([1, J], F32)
            nc.scalar.activation(out=sd, in_=var, func=AF.Sqrt, bias=epssb,
                                 scale=1.0)
            rstd = small.tile([1, J], F32)
            nc.vector.reciprocal(out=rstd, in_=sd)
            bc = small.tile([1, 2 * J], F32)
            nc.vector.tensor_tensor(out=bc[:, 0:J], in0=bnwsb, in1=rstd,
                                    op=OP.mult)
            bhat = small.tile([1, J], F32)
            nc.vector.tensor_tensor(out=bhat, in0=mu, in1=bc[:, 0:J],
                                    op=OP.mult)
            nc.vector.tensor_tensor(out=bc[:, J:2 * J], in0=bnbsb, in1=bhat,
                                    op=OP.subtract)

            ps_b = psp.tile([128, 2 * J], F32, name="ps_b", tag="ps")
            nc.tensor.matmul(ps_b, ones_row, bc, start=True, stop=True)
            rep = consts.tile([128, 2 * J], F32)
            nc.scalar.copy(out=rep, in_=ps_b)
            srb = consts.tile([128, J], BF16)
            nc.scalar.copy(out=srb, in_=ps_b[:, 0:J])
            srep = srb
            bhrep = rep[:, J:2 * J]

            # ================= pass 1: apply =================
            for w in range(NW):
                if w == 0:
                    yrd = yrd0
                else:
                    yrd = {}
                    for q in range(2):
                        t = ysp.tile([128, J, W], BF16, name="yrd", tag="ys")
                        nc.sync.dma_start(out=t[:, 0:9, :],
                                          in_=y_hbm[w, q][:, 0:9, :])
                        nc.sync.dma_start(out=t[:, 9:J, :],
                                          in_=y_hbm[w, q][:, 9:J, :])
                        yrd[q] = t
                # software-pipelined: the gate multiply + store for joint j
                # are emitted 2 joints later so the attention-chain latency
                # overlaps the next joints' DVE work
                pend = {}

                def flush(j):
                    ob, attb = pend.pop(j)
                    for q in range(2):
                        nc.vector.tensor_tensor(
                            out=ob[:, q, :], in0=ob[:, q, :], in1=attb,
                            op=OP.mult)
                    nc.gpsimd.dma_start(out=out_t[w, j], in_=ob)

                for j in range(J):
                    ob = obp.tile([128, 2, W], BF16, name="ob", tag="ob")
                    for q in range(2):
                        nc.vector.scalar_tensor_tensor(
                            out=ob[:, q, :],
                            in0=yrd[q][:, j, :],
                            scalar=srep[:, j:j + 1],
                            in1=x_sb[w][:, q, j, :],
                            op0=OP.mult,
                            op1=OP.add,
                        )
                    nc.vector.tensor_scalar(
                        out=ob, in0=ob,
                        scalar1=bhrep[:, j:j + 1], scalar2=0.0,
                        op0=OP.add, op1=OP.max)
                    hps = psp.tile([H, W], F32, name="hps", tag="ps")
                    nc.tensor.matmul(hps, wa1sb[:, 0, :], ob[:, 0, :],
                                     start=True, stop=False)
                    nc.tensor.matmul(hps, wa1sb[:, 1, :], ob[:, 1, :],
                                     start=False, stop=True)
                    hbf = hp.tile([H, W], BF16, name="hbf", tag="h")
                    nc.scalar.activation(out=hbf, in_=hps, func=AF.Relu,
                                         bias=ab1sb, scale=1.0)
                    aps = psp.tile([128, W], F32, name="aps", tag="ps")
                    nc.tensor.matmul(aps, w2rsb, hbf, start=True, stop=True)
                    attb = abp.tile([128, W], BF16, name="attb", tag="attb")
                    nc.scalar.activation(out=attb, in_=aps, func=AF.Sigmoid,
                                         bias=ab2rsb, scale=1.0)
                    pend[j] = (ob, attb)
                    if j >= 2:
                        flush(j - 2)
                for j in (J - 2, J - 1):
                    flush(j)

    nc.compile()
    return nc


_CACHE: dict = {}


def _host_inputs(x, U_w, U_b, V_w, V_b, bn_w, bn_b, att_w1, att_b1, att_w2,
                 att_b2):
    f32 = np.float32
    bf16 = ml_dtypes.bfloat16

    def chunks(wT):  # [C(in), M(out)] -> [a, q, 128, 128]
        return np.ascontiguousarray(
            wT.reshape(2, 128, 2, 128).transpose(0, 2, 1, 3))

    uT = np.ascontiguousarray(U_w.T).astype(f32)   # [c_in, c_out]
    vT = np.ascontiguousarray(V_w.T).astype(f32)
    wu = chunks(uT).astype(bf16)
    wv = np.stack([chunks(s * vT) for s in _VARIANTS]).astype(bf16)
    wa1 = np.ascontiguousarray(att_w1.T.reshape(2, 128, H)).astype(bf16)
    w2r = np.ascontiguousarray(
        np.tile(att_w2.T.astype(f32), (1, 128))).astype(bf16)  # [H, 128]
    rowsum = _ADJ.sum(axis=1)
    b2 = (rowsum[None, :] * V_b[:, None] + U_b[:, None]).astype(f32)  # [C, J]
    b2 = np.ascontiguousarray(b2.reshape(2, 128, J).transpose(1, 0, 2))
    bnw = bn_w.reshape(1, J).astype(f32)
    bnb = bn_b.reshape(1, J).astype(f32)
    ab1 = att_b1.reshape(H, 1).astype(f32)
    ab2r = np.ascontiguousarray(
        np.tile(att_b2.reshape(1, 1).astype(f32), (128, 1)))

    shared = dict(wu=wu, wv=wv, wa1=wa1, w2r=w2r, b2=b2, bnw=bnw, bnb=bnb,
                  ab1=ab1, ab2r=ab2r)

    xtf = np.ascontiguousarray(x.transpose(3, 2, 0, 1))  # [C, J, B, T]
    in_maps = []
    for i in range(NCORES):
        xc = xtf[:, :, i * BPC:(i + 1) * BPC, :].reshape(C, J, NBT)
        xc = xc.reshape(2, 128, J, NW, W)
        xc = np.ascontiguousarray(xc.transpose(3, 1, 0, 2, 4)).astype(bf16)
        in_maps.append(dict(xt=xc, **shared))
    return in_maps


def kernel(x, U_w, U_b, V_w, V_b, bn_w, bn_b, att_w1, att_b1, att_w2, att_b2,
           _trace=False):
    x = np.asarray(x, dtype=np.float32)
    args = [np.asarray(a, dtype=np.float32)
            for a in (U_w, U_b, V_w, V_b, bn_w, bn_b, att_w1, att_b1, att_w2,
                      att_b2)]
    in_maps = _host_inputs(x, *args)

    if "nc" not in _CACHE:
        _CACHE["nc"] = _build_program()
    nc = _CACHE["nc"]

    res = run_bass_kernel_spmd(nc, in_maps, list(range(NCORES)), trace=_trace)
    _CACHE["last_results"] = res

    # out_t per core: [NW, J, 128, 2, W] -> [BPC, T, J, C]
    outs = []
    for i in range(NCORES):
        o = res.results[i]["out_t"]                     # bf16
        o = o.transpose(3, 2, 1, 0, 4).reshape(C, J, NBT)
        o = o.transpose(2, 1, 0).reshape(BPC, T, J, C)
        outs.append(o)
    out = np.concatenate(outs, axis=0).astype(np.float32)
    return np.ascontiguousarray(out)
